# revision 7
# baseline (speedup 1.0000x reference)
"""FP8Linear Trainium2 kernel.

Computes out = quant_e4m3(x) @ quant_e4m3(w).T in fp32, distributed over 8
NeuronCores as a 2x4 grid (x rows x w rows). Per core:

  xT_in [2048, 4096] f32, wT_in [2048, 2048] f32 -> out [4096, 2048] f32

The host pre-transposes both operands (a layout/sharding choice), so the
contraction dim c is already the partition dim on-chip and no transposes
run on the device at all:

  loads: f32 c-chunk slabs -> ACT quantize f32->fp8e4 (the rounding step)
     into resident fp8 tiles XT [128c, 16s, 4096m], WT [128c, 16s, 2048o]
  matmul: fp8 DoubleRow (c-chunk pairs on partitions), fp32 PSUM, N=512
     tiles, 8 PSUM banks, ACT/DVE drains, 512 KiB output DMAs.

Fill-phase shape: the first output window can only complete once a full
contraction column (all 16 c-chunks) of w and x is resident, so w is
loaded in o-halves: the first burst needs w[:, :1024] + x[:, :512] only
(12 MiB), and the o-half matmul groups chase the c-ascending load order.
Loads alternate between the SP (sync) HWDGE FIFO and the SWDGE queue;
stores alternate between the ACT HWDGE FIFO and the SWDGE queue.

TRN fp8e4 (max 240) matches OCP e4m3fn on [0, 240]; inputs are randn-scale
so the quantization grid is identical to the jax reference. Scales are
applied on the host (exact for any scale: round(x*s) then /(s_in*s_w)).
"""

import numpy as np
import ml_dtypes

# ---- problem constants (hardcoded per task contract) ----
A_DIM, B_DIM, C_DIM, OUT_DIM = 4, 2048, 2048, 8192
M_FULL = A_DIM * B_DIM  # 8192
GRID_M, GRID_O = 2, 4
N_CORES = GRID_M * GRID_O
M_CORE = M_FULL // GRID_M   # 4096
O_CORE = OUT_DIM // GRID_O  # 2048

P = 128


def build_nc(m_core=M_CORE, o_core=O_CORE, c_dim=C_DIM,
             m_slab=512, n_tile=512, mm_psum_bufs=8):
    """Build the single-core Bass program (same program runs SPMD on 8 cores)."""
    import contextlib

    import concourse.bacc as bacc
    import concourse.mybir as mybir
    import concourse.tile as tile

    f32 = mybir.dt.float32
    fp8 = mybir.dt.float8e4
    Copy = mybir.ActivationFunctionType.Copy
    DR = mybir.MatmulPerfMode.DoubleRow

    S = c_dim // P              # c-chunks (16)
    SP = S // 2                 # DoubleRow pairs (8)
    MG = m_core // m_slab       # x slab groups (8)
    MWG = m_slab // P           # m windows per slab group (4)
    NT = o_core // n_tile       # o tiles (4)
    NTH = NT // 2               # o tiles per half (2)
    o_half = o_core // 2        # 1024

    nc = bacc.Bacc(None, target_bir_lowering=False, debug=False)
    x_in = nc.declare_dram_parameter("x_in", [c_dim, m_core], f32, isOutput=False)
    w_in = nc.declare_dram_parameter("w_in", [c_dim, o_core], f32, isOutput=False)
    out = nc.declare_dram_parameter("out", [m_core, o_core], f32, isOutput=True)

    with tile.TileContext(nc) as tc:
        with contextlib.ExitStack() as ctx:
            wstg = ctx.enter_context(tc.tile_pool(name="wstg", bufs=3))
            xstg = ctx.enter_context(tc.tile_pool(name="xstg", bufs=4))
            xres = ctx.enter_context(tc.tile_pool(name="xres", bufs=1))
            wres = ctx.enter_context(tc.tile_pool(name="wres", bufs=1))
            mmp = ctx.enter_context(
                tc.tile_pool(name="mmp", bufs=mm_psum_bufs, space="PSUM"))
            osb = ctx.enter_context(tc.tile_pool(name="osb", bufs=4))

            # resident fp8 operands, c on partitions
            XT = xres.tile([P, S, m_core], fp8)
            WT = wres.tile([P, S, o_core], fp8)

            def w_slab(s, oh, q):
                """w.T block [s*128c, oh*1024o): load f32, quantize into WT."""
                o0 = oh * o_half
                wst = wstg.tile([P, o_half], f32, tag="wst", name="wst")
                src = w_in[s * P:(s + 1) * P, o0:o0 + o_half]
                if q == 0:
                    nc.sync.dma_start(out=wst[:], in_=src)
                else:
                    nc.gpsimd.dma_start(out=wst[:], in_=src)
                nc.scalar.activation(WT[:, s, o0:o0 + o_half], wst[:], Copy)

            def x_slab(s, mg, q):
                """x.T block [s*128c, mg*m_slab m): load f32, quantize into XT."""
                m0 = mg * m_slab
                xst = xstg.tile([P, m_slab], f32, tag="xst", name="xst")
                src = x_in[s * P:(s + 1) * P, m0:m0 + m_slab]
                if q == 0:
                    nc.sync.dma_start(out=xst[:], in_=src)
                else:
                    nc.gpsimd.dma_start(out=xst[:], in_=src)
                nc.scalar.activation(XT[:, s, m0:m0 + m_slab], xst[:], Copy)

            def mm_half(mw, oh):
                """One 128-row m window x one 1024-col o half: 8 sp x 2 nt
                DoubleRow matmuls, drain 2 psum tiles, 512 KiB store."""
                ps = [mmp.tile([P, n_tile], f32, tag="mm_psum", name="mm_psum")
                      for _ in range(NTH)]
                for sp in range(SP):
                    lhsT = XT[:, 2 * sp:2 * sp + 2, mw * P:(mw + 1) * P]
                    for j in range(NTH):
                        nt = NTH * oh + j
                        nc.tensor.matmul(
                            ps[j][:], lhsT,
                            WT[:, 2 * sp:2 * sp + 2, nt * n_tile:(nt + 1) * n_tile],
                            start=(sp == 0), stop=(sp == SP - 1),
                            perf_mode=DR)
                ot = osb.tile([P, o_half], f32, tag="ot", name="ot")
                for j in range(NTH):
                    dst = ot[:, j * n_tile:(j + 1) * n_tile]
                    if j % 2 == 0:
                        nc.scalar.activation(dst, ps[j][:], Copy)
                    else:
                        nc.vector.tensor_copy(out=dst, in_=ps[j][:])
                dst = out[mw * P:(mw + 1) * P, oh * o_half:(oh + 1) * o_half]
                if (mw + oh) % 2 == 0:
                    nc.scalar.dma_start(out=dst, in_=ot[:])
                else:
                    nc.gpsimd.dma_start(out=dst, in_=ot[:])

            # ---- wave A: first o-half of w + first x slab group, c-ascending,
            # split across both load queues; the first matmul burst chases it ----
            for s in range(S):
                w_slab(s, 0, q=s % 2)
                x_slab(s, 0, q=(s + 1) % 2)
            for mw in range(0, MWG):
                mm_half(mw, 0)

            # ---- wave B: second o-half of w + second x slab group ----
            for s in range(S):
                w_slab(s, 1, q=s % 2)
                x_slab(s, 1, q=(s + 1) % 2)
            for mw in range(MWG, 2 * MWG):
                mm_half(mw, 0)
            for s in range(S):
                x_slab(s, 2, q=s % 2)
            for mw in range(0, MWG):
                mm_half(mw, 1)
            for mw in range(MWG, 2 * MWG):
                mm_half(mw, 1)
            for s in range(S):
                x_slab(s, 3, q=(s + 1) % 2)

            # ---- steady state: both halves per window, prefetch 2 groups out ----
            for mg in range(2, MG):
                for mw in range(mg * MWG, (mg + 1) * MWG):
                    mm_half(mw, 0)
                    mm_half(mw, 1)
                if mg + 2 < MG:
                    for s in range(S):
                        x_slab(s, mg + 2, q=(s + mg) % 2)

    nc.finalize()
    return nc


_NC = None


def _get_nc():
    global _NC
    if _NC is None:
        _NC = build_nc()
    return _NC


def kernel(input, weight, input_scale_e4m3=None, weight_scale_e4m3=None,
           **_unused):
    from concourse.bass_utils import run_bass_kernel_spmd

    x = np.asarray(input, dtype=np.float32).reshape(M_FULL, C_DIM)
    w = np.asarray(weight, dtype=np.float32)
    s_in = float(np.asarray(input_scale_e4m3)) if input_scale_e4m3 is not None else 1.0
    s_w = float(np.asarray(weight_scale_e4m3)) if weight_scale_e4m3 is not None else 1.0

    # reference semantics: round(x*s)/s etc.; fold scales on host (exact)
    if s_in != 1.0:
        x = x * s_in
    if s_w != 1.0:
        w = w * s_w

    # host-side layout staging: both operands transposed so c is the
    # on-chip partition (contraction) dim
    xT = [np.ascontiguousarray(x[mi * M_CORE:(mi + 1) * M_CORE].T)
          for mi in range(GRID_M)]
    wT = [np.ascontiguousarray(w[oj * O_CORE:(oj + 1) * O_CORE].T)
          for oj in range(GRID_O)]

    nc = _get_nc()
    in_maps = []
    for mi in range(GRID_M):
        for oj in range(GRID_O):
            in_maps.append({"x_in": xT[mi], "w_in": wT[oj]})
    res = run_bass_kernel_spmd(nc, in_maps, core_ids=list(range(N_CORES)))

    out = np.empty((M_FULL, OUT_DIM), np.float32)
    for k, r in enumerate(res.results):
        mi, oj = divmod(k, GRID_O)
        out[mi * M_CORE:(mi + 1) * M_CORE, oj * O_CORE:(oj + 1) * O_CORE] = r["out"]

    inv = 1.0 / (s_in * s_w)
    if inv != 1.0:
        out = out * inv
    return out.reshape(A_DIM, B_DIM, OUT_DIM)


# revision 8
# speedup vs baseline: 1.0006x; 1.0006x over previous
"""FP8Linear Trainium2 kernel.

Computes out = quant_e4m3(x) @ quant_e4m3(w).T in fp32, distributed over 8
NeuronCores as a 2x4 grid (x rows x w rows). Per core:

  xT_in [2048, 4096] bf16, wT_in [2048, 2048] bf16 -> out [4096, 2048] f32

Host-side staging (layout + lossless re-encoding, exact):
  - both operands transposed so the contraction dim c is the on-chip
    partition dim (no transposes run on the device at all);
  - f32 -> bf16 with ROUND-TO-ODD (truncate + sticky bit into the lsb).
    RNE(bf16_RO(v) -> e4m3) == RNE(v -> e4m3) exactly (double-rounding is
    exact when the intermediate format has >= 2 more mantissa bits than
    the target: bf16 has 8, e4m3 needs 3+2), so the device's ACT
    bf16->fp8 quantize reproduces the reference f32->fp8 bit-for-bit
    while input DMA traffic halves.

Device pipeline:
  loads: bf16 c-chunk slabs -> ACT quantize bf16->fp8e4 (the rounding
     step) into resident fp8 tiles XT [128c, 16s, 4096m],
     WT [128c, 16s, 2048o]
  matmul: fp8 DoubleRow (c-chunk pairs on partitions), fp32 PSUM, N=512
     tiles, 8 PSUM banks, ACT/DVE drains, 512 KiB f32 output DMAs.

Schedule: w's first o-half + x slab group 0 load first (6 MiB) and the
first matmul group chases them c-ascending; w's second o-half trickles
in behind x group 2; the o-half-1 matmul groups are emitted late, when
everything they need is already resident. Loads ride the two HWDGE
FIFOs (sync + scalar), stores ride the SWDGE queue — no FIFO ever mixes
loads and stores, so neither can head-of-line block the other.
"""

import numpy as np
import ml_dtypes

# ---- problem constants (hardcoded per task contract) ----
A_DIM, B_DIM, C_DIM, OUT_DIM = 4, 2048, 2048, 8192
M_FULL = A_DIM * B_DIM  # 8192
GRID_M, GRID_O = 2, 4
N_CORES = GRID_M * GRID_O
M_CORE = M_FULL // GRID_M   # 4096
O_CORE = OUT_DIM // GRID_O  # 2048

P = 128


def build_nc(m_core=M_CORE, o_core=O_CORE, c_dim=C_DIM,
             m_slab=512, n_tile=512, mm_psum_bufs=8):
    """Build the single-core Bass program (same program runs SPMD on 8 cores)."""
    import contextlib

    import concourse.bacc as bacc
    import concourse.mybir as mybir
    import concourse.tile as tile

    bf16 = mybir.dt.bfloat16
    f32 = mybir.dt.float32
    fp8 = mybir.dt.float8e4
    Copy = mybir.ActivationFunctionType.Copy
    DR = mybir.MatmulPerfMode.DoubleRow

    S = c_dim // P              # c-chunks (16)
    SP = S // 2                 # DoubleRow pairs (8)
    MG = m_core // m_slab       # x slab groups (8)
    MWG = m_slab // P           # m windows per slab group (4)
    NT = o_core // n_tile       # o tiles (4)
    NTH = NT // 2               # o tiles per half (2)
    o_half = o_core // 2        # 1024

    nc = bacc.Bacc(None, target_bir_lowering=False, debug=False)
    x_in = nc.declare_dram_parameter("x_in", [c_dim, m_core], bf16, isOutput=False)
    w_in = nc.declare_dram_parameter("w_in", [c_dim, o_core], bf16, isOutput=False)
    out = nc.declare_dram_parameter("out", [m_core, o_core], f32, isOutput=True)

    with tile.TileContext(nc) as tc:
        with contextlib.ExitStack() as ctx:
            wstg = ctx.enter_context(tc.tile_pool(name="wstg", bufs=3))
            xstg = ctx.enter_context(tc.tile_pool(name="xstg", bufs=4))
            xres = ctx.enter_context(tc.tile_pool(name="xres", bufs=1))
            wres = ctx.enter_context(tc.tile_pool(name="wres", bufs=1))
            mmp = ctx.enter_context(
                tc.tile_pool(name="mmp", bufs=mm_psum_bufs, space="PSUM"))
            osb = ctx.enter_context(tc.tile_pool(name="osb", bufs=4))

            # resident fp8 operands, c on partitions
            XT = xres.tile([P, S, m_core], fp8)
            WT = wres.tile([P, S, o_core], fp8)

            def w_slab(s, oh, q):
                """w.T block [s*128c, oh*1024o): load bf16, quantize into WT."""
                o0 = oh * o_half
                wst = wstg.tile([P, o_half], bf16, tag="wst", name="wst")
                src = w_in[s * P:(s + 1) * P, o0:o0 + o_half]
                if q == 0:
                    nc.sync.dma_start(out=wst[:], in_=src)
                else:
                    nc.scalar.dma_start(out=wst[:], in_=src)
                nc.scalar.activation(WT[:, s, o0:o0 + o_half], wst[:], Copy)

            def x_slab(s, mg, q):
                """x.T block [s*128c, mg*m_slab m): load bf16, quantize into XT."""
                m0 = mg * m_slab
                xst = xstg.tile([P, m_slab], bf16, tag="xst", name="xst")
                src = x_in[s * P:(s + 1) * P, m0:m0 + m_slab]
                if q == 0:
                    nc.sync.dma_start(out=xst[:], in_=src)
                else:
                    nc.scalar.dma_start(out=xst[:], in_=src)
                nc.scalar.activation(XT[:, s, m0:m0 + m_slab], xst[:], Copy)

            def mm_half(mw, oh):
                """One 128-row m window x one 1024-col o half: 8 sp x 2 nt
                DoubleRow matmuls, drain 2 psum tiles, 512 KiB store."""
                ps = [mmp.tile([P, n_tile], f32, tag="mm_psum", name="mm_psum")
                      for _ in range(NTH)]
                for sp in range(SP):
                    lhsT = XT[:, 2 * sp:2 * sp + 2, mw * P:(mw + 1) * P]
                    for j in range(NTH):
                        nt = NTH * oh + j
                        nc.tensor.matmul(
                            ps[j][:], lhsT,
                            WT[:, 2 * sp:2 * sp + 2, nt * n_tile:(nt + 1) * n_tile],
                            start=(sp == 0), stop=(sp == SP - 1),
                            perf_mode=DR)
                ot = osb.tile([P, o_half], f32, tag="ot", name="ot")
                for j in range(NTH):
                    dst = ot[:, j * n_tile:(j + 1) * n_tile]
                    if j % 2 == 0:
                        nc.scalar.activation(dst, ps[j][:], Copy)
                    else:
                        nc.vector.tensor_copy(out=dst, in_=ps[j][:])
                nc.gpsimd.dma_start(
                    out=out[mw * P:(mw + 1) * P, oh * o_half:(oh + 1) * o_half],
                    in_=ot[:])

            def mg_windows(mg):
                return range(mg * MWG, (mg + 1) * MWG)

            # ---- prefix: first o-half of w + x group 0, c-ascending on both
            # HWDGE queues; the first matmul group chases them ----
            for s in range(S):
                w_slab(s, 0, q=s % 2)
                x_slab(s, 0, q=(s + 1) % 2)
            for mw in mg_windows(0):
                mm_half(mw, 0)

            # ---- stream x groups; w's second o-half trickles behind group 2;
            # o-half-1 matmul groups backfill once everything is resident ----
            for s in range(S):
                x_slab(s, 1, q=s % 2)
            for mw in mg_windows(1):
                mm_half(mw, 0)
            for s in range(S):
                x_slab(s, 2, q=(s + 1) % 2)
                w_slab(s, 1, q=s % 2)
            for mw in mg_windows(2):
                mm_half(mw, 0)
            for mg in range(3, MG):
                for s in range(S):
                    x_slab(s, mg, q=(s + mg) % 2)
                for mw in mg_windows(mg):
                    mm_half(mw, 0)
                if mg >= 4:
                    for mw in mg_windows(mg - 4):
                        mm_half(mw, 1)
            for mg in range(MG - 4, MG):
                for mw in mg_windows(mg):
                    mm_half(mw, 1)

    nc.finalize()
    return nc


def _round_to_odd_bf16(a):
    """f32 -> bf16 by truncation with the sticky bit ORed into the lsb.

    RNE(result -> e4m3) == RNE(a -> e4m3) exactly (no double rounding).
    """
    u = np.ascontiguousarray(a, dtype=np.float32).view(np.uint32)
    hi = (u >> 16).astype(np.uint16)
    hi |= ((u & 0xFFFF) != 0).astype(np.uint16)
    return hi.view(ml_dtypes.bfloat16)


_NC = None


def _get_nc():
    global _NC
    if _NC is None:
        _NC = build_nc()
    return _NC


def kernel(input, weight, input_scale_e4m3=None, weight_scale_e4m3=None,
           **_unused):
    from concourse.bass_utils import run_bass_kernel_spmd

    x = np.asarray(input, dtype=np.float32).reshape(M_FULL, C_DIM)
    w = np.asarray(weight, dtype=np.float32)
    s_in = float(np.asarray(input_scale_e4m3)) if input_scale_e4m3 is not None else 1.0
    s_w = float(np.asarray(weight_scale_e4m3)) if weight_scale_e4m3 is not None else 1.0

    # reference semantics: round(x*s)/s etc.; fold scales on host (exact)
    if s_in != 1.0:
        x = x * s_in
    if s_w != 1.0:
        w = w * s_w

    # host-side staging: round-to-odd bf16 (exact w.r.t. the later fp8
    # RNE quantize) + transpose so c is the on-chip contraction dim
    xb = _round_to_odd_bf16(x)
    wb = _round_to_odd_bf16(w)
    xT = [np.ascontiguousarray(xb[mi * M_CORE:(mi + 1) * M_CORE].T)
          for mi in range(GRID_M)]
    wT = [np.ascontiguousarray(wb[oj * O_CORE:(oj + 1) * O_CORE].T)
          for oj in range(GRID_O)]

    nc = _get_nc()
    in_maps = []
    for mi in range(GRID_M):
        for oj in range(GRID_O):
            in_maps.append({"x_in": xT[mi], "w_in": wT[oj]})
    res = run_bass_kernel_spmd(nc, in_maps, core_ids=list(range(N_CORES)))

    out = np.empty((M_FULL, OUT_DIM), np.float32)
    for k, r in enumerate(res.results):
        mi, oj = divmod(k, GRID_O)
        out[mi * M_CORE:(mi + 1) * M_CORE, oj * O_CORE:(oj + 1) * O_CORE] = r["out"]

    inv = 1.0 / (s_in * s_w)
    if inv != 1.0:
        out = out * inv
    return out.reshape(A_DIM, B_DIM, OUT_DIM)


# revision 10
# speedup vs baseline: 1.0250x; 1.0245x over previous
"""FP8Linear Trainium2 kernel.

Computes out = quant_e4m3(x) @ quant_e4m3(w).T in fp32, distributed over 8
NeuronCores as a 2x4 grid (x rows x w rows). Per core:

  xT_in [2048, 4096] bf16, wT_in [2048, 2048] bf16 -> out [4096, 2048] f32

Host-side staging (layout + lossless re-encoding, exact):
  - both operands transposed so the contraction dim c is the on-chip
    partition dim (no transposes run on the device at all);
  - f32 -> bf16 with ROUND-TO-ODD (truncate + sticky bit into the lsb).
    RNE(bf16_RO(v) -> e4m3) == RNE(v -> e4m3) exactly (double-rounding is
    exact when the intermediate format has >= 2 more mantissa bits than
    the target: bf16 has 8, e4m3 needs 3+2), so the device's ACT
    bf16->fp8 quantize reproduces the reference f32->fp8 bit-for-bit
    while input DMA traffic halves.

Device pipeline:
  loads: bf16 c-chunk slabs -> ACT quantize bf16->fp8e4 (the rounding
     step) into resident fp8 tiles XT [128c, 16s, 4096m],
     WT [128c, 16s, 2048o]
  matmul: fp8 DoubleRow (c-chunk pairs on partitions), fp32 PSUM, N=512
     tiles, 8 PSUM banks, ACT/DVE drains, 512 KiB f32 output DMAs.

Schedule: w's first o-half + x slab group 0 load first (6 MiB) and the
first matmul group chases them c-ascending; w's second o-half trickles
in behind x group 2; the o-half-1 matmul groups are emitted late, when
everything they need is already resident. Loads ride the two HWDGE
FIFOs (sync + scalar), stores ride the SWDGE queue — no FIFO ever mixes
loads and stores, so neither can head-of-line block the other.
"""

import numpy as np
import ml_dtypes

# ---- problem constants (hardcoded per task contract) ----
A_DIM, B_DIM, C_DIM, OUT_DIM = 4, 2048, 2048, 8192
M_FULL = A_DIM * B_DIM  # 8192
GRID_M, GRID_O = 2, 4
N_CORES = GRID_M * GRID_O
M_CORE = M_FULL // GRID_M   # 4096
O_CORE = OUT_DIM // GRID_O  # 2048

P = 128


def build_nc(m_core=M_CORE, o_core=O_CORE, c_dim=C_DIM,
             m_slab=512, n_tile=512, mm_psum_bufs=8):
    """Build the single-core Bass program (same program runs SPMD on 8 cores)."""
    import contextlib

    import concourse.bacc as bacc
    import concourse.mybir as mybir
    import concourse.tile as tile

    bf16 = mybir.dt.bfloat16
    f32 = mybir.dt.float32
    fp8 = mybir.dt.float8e4
    Copy = mybir.ActivationFunctionType.Copy
    DR = mybir.MatmulPerfMode.DoubleRow

    S = c_dim // P              # c-chunks (16)
    SP = S // 2                 # DoubleRow pairs (8)
    MG = m_core // m_slab       # x slab groups (8)
    MWG = m_slab // P           # m windows per slab group (4)
    NT = o_core // n_tile       # o tiles (4)
    NTH = NT // 2               # o tiles per half (2)
    o_half = o_core // 2        # 1024

    nc = bacc.Bacc(None, target_bir_lowering=False, debug=False)
    x_in = nc.declare_dram_parameter("x_in", [c_dim, m_core], bf16, isOutput=False)
    w_in = nc.declare_dram_parameter("w_in", [c_dim, o_core], bf16, isOutput=False)
    out = nc.declare_dram_parameter("out", [m_core, o_core], f32, isOutput=True)

    with tile.TileContext(nc) as tc:
        with contextlib.ExitStack() as ctx:
            wstg = ctx.enter_context(tc.tile_pool(name="wstg", bufs=3))
            xstg = ctx.enter_context(tc.tile_pool(name="xstg", bufs=4))
            xres = ctx.enter_context(tc.tile_pool(name="xres", bufs=1))
            wres = ctx.enter_context(tc.tile_pool(name="wres", bufs=1))
            mmp = ctx.enter_context(
                tc.tile_pool(name="mmp", bufs=mm_psum_bufs, space="PSUM"))
            osb = ctx.enter_context(tc.tile_pool(name="osb", bufs=4))

            # resident fp8 operands, c on partitions
            XT = xres.tile([P, S, m_core], fp8)
            WT = wres.tile([P, S, o_core], fp8)

            def w_slab(s, oh, q):
                """w.T block [s*128c, oh*1024o): load bf16, quantize into WT."""
                o0 = oh * o_half
                wst = wstg.tile([P, o_half], bf16, tag="wst", name="wst")
                src = w_in[s * P:(s + 1) * P, o0:o0 + o_half]
                if q == 0:
                    nc.sync.dma_start(out=wst[:], in_=src)
                else:
                    nc.scalar.dma_start(out=wst[:], in_=src)
                nc.scalar.activation(WT[:, s, o0:o0 + o_half], wst[:], Copy)

            def x_slab(s, mg, q):
                """x.T block [s*128c, mg*m_slab m): load bf16, quantize into XT.
                Quantizes alternate between ACT and DVE so neither engine's
                FIFO paces the matmul chase."""
                m0 = mg * m_slab
                xst = xstg.tile([P, m_slab], bf16, tag="xst", name="xst")
                src = x_in[s * P:(s + 1) * P, m0:m0 + m_slab]
                if q == 0:
                    nc.sync.dma_start(out=xst[:], in_=src)
                else:
                    nc.scalar.dma_start(out=xst[:], in_=src)
                dst = XT[:, s, m0:m0 + m_slab]
                if s % 2 == 0:
                    nc.scalar.activation(dst, xst[:], Copy)
                else:
                    nc.vector.tensor_copy(out=dst, in_=xst[:])

            def mm_half(mw, oh):
                """One 128-row m window x one 1024-col o half: 8 sp x 2 nt
                DoubleRow matmuls, drain 2 psum tiles, 512 KiB store."""
                ps = [mmp.tile([P, n_tile], f32, tag="mm_psum", name="mm_psum")
                      for _ in range(NTH)]
                for sp in range(SP):
                    lhsT = XT[:, 2 * sp:2 * sp + 2, mw * P:(mw + 1) * P]
                    for j in range(NTH):
                        nt = NTH * oh + j
                        nc.tensor.matmul(
                            ps[j][:], lhsT,
                            WT[:, 2 * sp:2 * sp + 2, nt * n_tile:(nt + 1) * n_tile],
                            start=(sp == 0), stop=(sp == SP - 1),
                            perf_mode=DR)
                ot = osb.tile([P, o_half], f32, tag="ot", name="ot")
                for j in range(NTH):
                    dst = ot[:, j * n_tile:(j + 1) * n_tile]
                    nc.vector.tensor_copy(out=dst, in_=ps[j][:])
                nc.gpsimd.dma_start(
                    out=out[mw * P:(mw + 1) * P, oh * o_half:(oh + 1) * o_half],
                    in_=ot[:])

            def mg_windows(mg):
                return range(mg * MWG, (mg + 1) * MWG)

            # ---- prefix: first o-half of w + x group 0, c-ascending on both
            # HWDGE queues; the first matmul group chases them ----
            for s in range(S):
                w_slab(s, 0, q=s % 2)
                x_slab(s, 0, q=(s + 1) % 2)
            for mw in mg_windows(0):
                mm_half(mw, 0)

            # ---- stream x groups; w's second o-half trickles behind group 2;
            # o-half-1 matmul groups backfill once everything is resident ----
            for s in range(S):
                x_slab(s, 1, q=s % 2)
            for mw in mg_windows(1):
                mm_half(mw, 0)
            for s in range(S):
                x_slab(s, 2, q=(s + 1) % 2)
                w_slab(s, 1, q=s % 2)
            for mw in mg_windows(2):
                mm_half(mw, 0)
            for mg in range(3, MG):
                for s in range(S):
                    x_slab(s, mg, q=(s + mg) % 2)
                for mw in mg_windows(mg):
                    mm_half(mw, 0)
                if mg >= 4:
                    for mw in mg_windows(mg - 4):
                        mm_half(mw, 1)
            for mg in range(MG - 4, MG):
                for mw in mg_windows(mg):
                    mm_half(mw, 1)

    nc.finalize()
    return nc


def _round_to_odd_bf16(a):
    """f32 -> bf16 by truncation with the sticky bit ORed into the lsb.

    RNE(result -> e4m3) == RNE(a -> e4m3) exactly (no double rounding).
    """
    u = np.ascontiguousarray(a, dtype=np.float32).view(np.uint32)
    hi = (u >> 16).astype(np.uint16)
    hi |= ((u & 0xFFFF) != 0).astype(np.uint16)
    return hi.view(ml_dtypes.bfloat16)


_NC = None


def _get_nc():
    global _NC
    if _NC is None:
        _NC = build_nc()
    return _NC


def kernel(input, weight, input_scale_e4m3=None, weight_scale_e4m3=None,
           **_unused):
    from concourse.bass_utils import run_bass_kernel_spmd

    x = np.asarray(input, dtype=np.float32).reshape(M_FULL, C_DIM)
    w = np.asarray(weight, dtype=np.float32)
    s_in = float(np.asarray(input_scale_e4m3)) if input_scale_e4m3 is not None else 1.0
    s_w = float(np.asarray(weight_scale_e4m3)) if weight_scale_e4m3 is not None else 1.0

    # reference semantics: round(x*s)/s etc.; fold scales on host (exact)
    if s_in != 1.0:
        x = x * s_in
    if s_w != 1.0:
        w = w * s_w

    # host-side staging: round-to-odd bf16 (exact w.r.t. the later fp8
    # RNE quantize) + transpose so c is the on-chip contraction dim
    xb = _round_to_odd_bf16(x)
    wb = _round_to_odd_bf16(w)
    xT = [np.ascontiguousarray(xb[mi * M_CORE:(mi + 1) * M_CORE].T)
          for mi in range(GRID_M)]
    wT = [np.ascontiguousarray(wb[oj * O_CORE:(oj + 1) * O_CORE].T)
          for oj in range(GRID_O)]

    nc = _get_nc()
    in_maps = []
    for mi in range(GRID_M):
        for oj in range(GRID_O):
            in_maps.append({"x_in": xT[mi], "w_in": wT[oj]})
    res = run_bass_kernel_spmd(nc, in_maps, core_ids=list(range(N_CORES)))

    out = np.empty((M_FULL, OUT_DIM), np.float32)
    for k, r in enumerate(res.results):
        mi, oj = divmod(k, GRID_O)
        out[mi * M_CORE:(mi + 1) * M_CORE, oj * O_CORE:(oj + 1) * O_CORE] = r["out"]

    inv = 1.0 / (s_in * s_w)
    if inv != 1.0:
        out = out * inv
    return out.reshape(A_DIM, B_DIM, OUT_DIM)


# revision 11
# speedup vs baseline: 1.1181x; 1.0908x over previous
"""FP8Linear Trainium2 kernel.

Computes out = quant_e4m3(x) @ quant_e4m3(w).T in fp32, distributed over 8
NeuronCores as a 2x4 grid (x rows x w rows). Per core:

  xT_in [2048, 4096] bf16, wT_in [2048, 2048] bf16 -> out [4096, 2048] f32

Host-side staging (layout + lossless re-encoding, exact):
  - both operands transposed so the contraction dim c is the on-chip
    partition dim (no transposes run on the device at all);
  - f32 -> bf16 with ROUND-TO-ODD (truncate + sticky bit into the lsb).
    RNE(bf16_RO(v) -> e4m3) == RNE(v -> e4m3) exactly (double-rounding is
    exact when the intermediate format has >= 2 more mantissa bits than
    the target: bf16 has 8, e4m3 needs 3+2), so the device's ACT
    bf16->fp8 quantize reproduces the reference f32->fp8 bit-for-bit
    while input DMA traffic halves.

Device pipeline:
  loads: bf16 c-chunk slabs -> ACT quantize bf16->fp8e4 (the rounding
     step) into resident fp8 tiles XT [128c, 16s, 4096m],
     WT [128c, 16s, 2048o]
  matmul: fp8 DoubleRow (c-chunk pairs on partitions), fp32 PSUM, N=512
     tiles, 8 PSUM banks, ACT/DVE drains, 512 KiB f32 output DMAs.

Schedule: w's first o-half + x slab group 0 load first (6 MiB) and the
first matmul group chases them c-ascending; w's second o-half trickles
in behind x group 2; the o-half-1 matmul groups are emitted late, when
everything they need is already resident. Loads ride the two HWDGE
FIFOs (sync + scalar), stores ride the SWDGE queue — no FIFO ever mixes
loads and stores, so neither can head-of-line block the other.
"""

import numpy as np
import ml_dtypes

# ---- problem constants (hardcoded per task contract) ----
A_DIM, B_DIM, C_DIM, OUT_DIM = 4, 2048, 2048, 8192
M_FULL = A_DIM * B_DIM  # 8192
GRID_M, GRID_O = 2, 4
N_CORES = GRID_M * GRID_O
M_CORE = M_FULL // GRID_M   # 4096
O_CORE = OUT_DIM // GRID_O  # 2048

P = 128


def build_nc(m_core=M_CORE, o_core=O_CORE, c_dim=C_DIM,
             m_slab=512, n_tile=512, mm_psum_bufs=8):
    """Build the single-core Bass program (same program runs SPMD on 8 cores)."""
    import contextlib

    import concourse.bacc as bacc
    import concourse.mybir as mybir
    import concourse.tile as tile

    bf16 = mybir.dt.bfloat16
    f32 = mybir.dt.float32
    fp8 = mybir.dt.float8e4
    Copy = mybir.ActivationFunctionType.Copy
    DR = mybir.MatmulPerfMode.DoubleRow

    S = c_dim // P              # c-chunks (16)
    SP = S // 2                 # DoubleRow pairs (8)
    MG = m_core // m_slab       # x slab groups (8)
    MWG = m_slab // P           # m windows per slab group (4)
    NT = o_core // n_tile       # o tiles (4)
    NTH = NT // 2               # o tiles per half (2)
    o_half = o_core // 2        # 1024

    nc = bacc.Bacc(None, target_bir_lowering=False, debug=False)
    x_in = nc.declare_dram_parameter("x_in", [c_dim, m_core], bf16, isOutput=False)
    w_in = nc.declare_dram_parameter("w_in", [c_dim, o_core], bf16, isOutput=False)
    out = nc.declare_dram_parameter("out", [m_core, o_core], f32, isOutput=True)

    with tile.TileContext(nc) as tc:
        with contextlib.ExitStack() as ctx:
            wstg = ctx.enter_context(tc.tile_pool(name="wstg", bufs=8))
            xstg = ctx.enter_context(tc.tile_pool(name="xstg", bufs=16))
            xres = ctx.enter_context(tc.tile_pool(name="xres", bufs=1))
            wres = ctx.enter_context(tc.tile_pool(name="wres", bufs=1))
            mmp = ctx.enter_context(
                tc.tile_pool(name="mmp", bufs=mm_psum_bufs, space="PSUM"))
            osb = ctx.enter_context(tc.tile_pool(name="osb", bufs=4))

            # resident fp8 operands, c on partitions
            XT = xres.tile([P, S, m_core], fp8)
            WT = wres.tile([P, S, o_core], fp8)

            def w_slab(s, oh, q):
                """w.T block [s*128c, oh*1024o): load bf16, quantize into WT."""
                o0 = oh * o_half
                wst = wstg.tile([P, o_half], bf16, tag="wst", name="wst")
                src = w_in[s * P:(s + 1) * P, o0:o0 + o_half]
                if q == 0:
                    nc.sync.dma_start(out=wst[:], in_=src)
                else:
                    nc.scalar.dma_start(out=wst[:], in_=src)
                nc.scalar.activation(WT[:, s, o0:o0 + o_half], wst[:], Copy)

            def x_slab(s, mg, q):
                """x.T block [s*128c, mg*m_slab m): load bf16, quantize into XT.
                Quantizes alternate between ACT and DVE so neither engine's
                FIFO paces the matmul chase."""
                m0 = mg * m_slab
                xst = xstg.tile([P, m_slab], bf16, tag="xst", name="xst")
                src = x_in[s * P:(s + 1) * P, m0:m0 + m_slab]
                if q == 0:
                    nc.sync.dma_start(out=xst[:], in_=src)
                else:
                    nc.scalar.dma_start(out=xst[:], in_=src)
                dst = XT[:, s, m0:m0 + m_slab]
                if s % 2 == 0:
                    nc.scalar.activation(dst, xst[:], Copy)
                else:
                    nc.vector.tensor_copy(out=dst, in_=xst[:])

            def mm_half(mw, oh):
                """One 128-row m window x one 1024-col o half: 8 sp x 2 nt
                DoubleRow matmuls, drain 2 psum tiles, 512 KiB store."""
                ps = [mmp.tile([P, n_tile], f32, tag="mm_psum", name="mm_psum")
                      for _ in range(NTH)]
                for sp in range(SP):
                    lhsT = XT[:, 2 * sp:2 * sp + 2, mw * P:(mw + 1) * P]
                    for j in range(NTH):
                        nt = NTH * oh + j
                        nc.tensor.matmul(
                            ps[j][:], lhsT,
                            WT[:, 2 * sp:2 * sp + 2, nt * n_tile:(nt + 1) * n_tile],
                            start=(sp == 0), stop=(sp == SP - 1),
                            perf_mode=DR)
                ot = osb.tile([P, o_half], f32, tag="ot", name="ot")
                for j in range(NTH):
                    dst = ot[:, j * n_tile:(j + 1) * n_tile]
                    nc.vector.tensor_copy(out=dst, in_=ps[j][:])
                nc.gpsimd.dma_start(
                    out=out[mw * P:(mw + 1) * P, oh * o_half:(oh + 1) * o_half],
                    in_=ot[:])

            def mg_windows(mg):
                return range(mg * MWG, (mg + 1) * MWG)

            # ---- prefix: first o-half of w + x group 0, c-ascending on both
            # HWDGE queues; the first matmul group chases them ----
            for s in range(S):
                w_slab(s, 0, q=s % 2)
                x_slab(s, 0, q=(s + 1) % 2)
            for mw in mg_windows(0):
                mm_half(mw, 0)

            # ---- stream x groups; w's second o-half trickles behind group 2;
            # o-half-1 matmul groups backfill once everything is resident ----
            for s in range(S):
                x_slab(s, 1, q=s % 2)
            for mw in mg_windows(1):
                mm_half(mw, 0)
            for s in range(S):
                x_slab(s, 2, q=(s + 1) % 2)
                w_slab(s, 1, q=s % 2)
            for mw in mg_windows(2):
                mm_half(mw, 0)
            for mg in range(3, MG):
                for s in range(S):
                    x_slab(s, mg, q=(s + mg) % 2)
                for mw in mg_windows(mg):
                    mm_half(mw, 0)
                if mg >= 4:
                    for mw in mg_windows(mg - 4):
                        mm_half(mw, 1)
            for mg in range(MG - 4, MG):
                for mw in mg_windows(mg):
                    mm_half(mw, 1)

    nc.finalize()
    return nc


def _round_to_odd_bf16(a):
    """f32 -> bf16 by truncation with the sticky bit ORed into the lsb.

    RNE(result -> e4m3) == RNE(a -> e4m3) exactly (no double rounding).
    """
    u = np.ascontiguousarray(a, dtype=np.float32).view(np.uint32)
    hi = (u >> 16).astype(np.uint16)
    hi |= ((u & 0xFFFF) != 0).astype(np.uint16)
    return hi.view(ml_dtypes.bfloat16)


_NC = None


def _get_nc():
    global _NC
    if _NC is None:
        _NC = build_nc()
    return _NC


def kernel(input, weight, input_scale_e4m3=None, weight_scale_e4m3=None,
           **_unused):
    from concourse.bass_utils import run_bass_kernel_spmd

    x = np.asarray(input, dtype=np.float32).reshape(M_FULL, C_DIM)
    w = np.asarray(weight, dtype=np.float32)
    s_in = float(np.asarray(input_scale_e4m3)) if input_scale_e4m3 is not None else 1.0
    s_w = float(np.asarray(weight_scale_e4m3)) if weight_scale_e4m3 is not None else 1.0

    # reference semantics: round(x*s)/s etc.; fold scales on host (exact)
    if s_in != 1.0:
        x = x * s_in
    if s_w != 1.0:
        w = w * s_w

    # host-side staging: round-to-odd bf16 (exact w.r.t. the later fp8
    # RNE quantize) + transpose so c is the on-chip contraction dim
    xb = _round_to_odd_bf16(x)
    wb = _round_to_odd_bf16(w)
    xT = [np.ascontiguousarray(xb[mi * M_CORE:(mi + 1) * M_CORE].T)
          for mi in range(GRID_M)]
    wT = [np.ascontiguousarray(wb[oj * O_CORE:(oj + 1) * O_CORE].T)
          for oj in range(GRID_O)]

    nc = _get_nc()
    in_maps = []
    for mi in range(GRID_M):
        for oj in range(GRID_O):
            in_maps.append({"x_in": xT[mi], "w_in": wT[oj]})
    res = run_bass_kernel_spmd(nc, in_maps, core_ids=list(range(N_CORES)))

    out = np.empty((M_FULL, OUT_DIM), np.float32)
    for k, r in enumerate(res.results):
        mi, oj = divmod(k, GRID_O)
        out[mi * M_CORE:(mi + 1) * M_CORE, oj * O_CORE:(oj + 1) * O_CORE] = r["out"]

    inv = 1.0 / (s_in * s_w)
    if inv != 1.0:
        out = out * inv
    return out.reshape(A_DIM, B_DIM, OUT_DIM)
